# revision 22
# baseline (speedup 1.0000x reference)
"""PlainGCN message passing on 8 TRN2 NeuronCores.

Computation (reference):
    deg = bincount(h); dis = deg**-0.5; norm = dis[t]*dis[h]
    out = relu(segment_sum(norm[:,None] * x[h], t, N))

Strategy (v2):
  - Fold dis[h] into x host-side: x2 = dis[:,None]*x (bf16). Then
    out[t] = relu(dis[t] * segment_sum(x2[h], t)) — the per-edge norm
    disappears; dis[t] is applied once per dest tile, fused with the
    ReLU on ScalarE (per-partition scale).
  - Shard edges by destination: core c owns dest nodes
    [c*N/8, (c+1)*N/8); x2 replicated in HBM.
  - Per (dest tile j, source bucket b) the edges form a run padded to
    a multiple of 128 slots (shared SPMD schedule = max over cores).
    Pad slots carry gather idx -1 (skipped by the DMA when trailing)
    and tloc -1 (one-hot row = 0).
  - dma_gather x2 rows (256 B bf16) in <=1024-idx chunks,
    single_packet=True, round-robin over 4 SWDGE queues. The gather is
    Q7 descriptor-generation bound (~4 ns/idx), so everything else is
    scheduled to hide under it.
  - Segment-sum per dest tile: one-hot(tloc)=is_equal(iota) on DVE
    (bf16), TensorE matmul accumulation of full 128-slot columns into
    one PSUM bank per tile, then Relu(dis_t * psum) on ScalarE, DMA out.
"""

import numpy as np

import concourse.bacc as bacc
import concourse.mybir as mybir
import concourse.tile as tile
from concourse.bass_utils import run_bass_kernel_spmd
from concourse.library_config import mlp as mlp_lib

P = 128
N_NODES = 100000
D_FEAT = 128
N_CORES = 8
BUCKET_W = 25000     # source-bucket width (< 32768 so idx fits int16)
TILE_BLOCK = 4       # dest tiles per gather block
GATHER_CHUNK = 1024  # max idxs per dma_gather (single_packet safe limit)


def _preprocess(x, h, t):
    n, d = x.shape
    assert (n, d) == (N_NODES, D_FEAT)
    npc = n // N_CORES
    n_tiles = -(-npc // P)
    bucket = BUCKET_W
    n_buckets = -(-n // bucket)

    h = h.astype(np.int64)
    t = t.astype(np.int64)

    deg = np.bincount(h, minlength=n).astype(np.float64)
    dis = np.where(deg > 0, deg, 1.0) ** -0.5
    x2 = (x.astype(np.float64) * dis[:, None]).astype(np.float32)

    core = t // npc
    tloc = t - core * npc
    j = tloc // P
    tin = (tloc % P).astype(np.float32)
    b = h // BUCKET_W
    gidx_all = (h - b * BUCKET_W).astype(np.int16)

    counts = np.zeros((N_CORES, n_tiles, n_buckets), dtype=np.int64)
    np.add.at(counts, (core, j, b), 1)
    run_len = counts.max(axis=0)
    run_len = -(-run_len // P) * P  # full 128-slot columns only

    # stream order: blocks of TILE_BLOCK dest tiles; per block, per bucket,
    # the tiles' runs back to back.
    n_blocks = -(-n_tiles // TILE_BLOCK)
    run_start = np.zeros((n_tiles, n_buckets), dtype=np.int64)
    gathers = []  # (bucket, start, length) — <=GATHER_CHUNK, 128-aligned
    pos = 0
    for blk in range(n_blocks):
        tiles_blk = range(blk * TILE_BLOCK, min((blk + 1) * TILE_BLOCK, n_tiles))
        for bb in range(n_buckets):
            s0 = pos
            for jj in tiles_blk:
                run_start[jj, bb] = pos
                pos += int(run_len[jj, bb])
            # split [s0, pos) into gather chunks at run boundaries
            c0 = s0
            for jj in tiles_blk:
                r = int(run_len[jj, bb])
                end = run_start[jj, bb] + r
                if end - c0 > GATHER_CHUNK:
                    if run_start[jj, bb] > c0:
                        gathers.append((bb, c0, int(run_start[jj, bb] - c0)))
                    c0 = int(run_start[jj, bb])
                    while end - c0 > GATHER_CHUNK:
                        gathers.append((bb, c0, GATHER_CHUNK))
                        c0 += GATHER_CHUNK
            if pos > c0:
                gathers.append((bb, c0, int(pos - c0)))
    e_pad = pos
    n_cols = e_pad // P
    assert all(ln <= GATHER_CHUNK and ln % P == 0 for (_b, _s, ln) in gathers)

    # per-tile column lists: (col, bucket)
    tile_cols = []
    for jj in range(n_tiles):
        cols = []
        for bb in range(n_buckets):
            s, r = int(run_start[jj, bb]), int(run_len[jj, bb])
            cols.extend((c, bb) for c in range(s // P, (s + r) // P))
        tile_cols.append(cols)

    # per-core streams
    order_key = (j // TILE_BLOCK) * (n_buckets * n_tiles) + b * n_tiles + j
    per_core = []
    for c in range(N_CORES):
        sel = np.nonzero(core == c)[0]
        sel = sel[np.argsort(order_key[sel], kind="stable")]
        jj = j[sel]
        bb2 = b[sel]
        key = jj * n_buckets + bb2
        change = np.r_[True, key[1:] != key[:-1]]
        grp_id = np.cumsum(change) - 1
        first_pos = np.nonzero(change)[0]
        within = np.arange(len(sel)) - first_pos[grp_id]
        posn = run_start[jj, bb2] + within

        gi = np.full(e_pad, -1, dtype=np.int16)
        tf = np.full(e_pad, -1.0, dtype=np.float32)
        gi[posn] = gidx_all[sel]
        tf[posn] = tin[sel]
        # Trailing pads of each gather chunk stay -1 and the per-core
        # stripped count goes in via num_idxs_reg — the NX ring
        # reservation and the Q7 -1-stripping then agree (they MUST, or
        # the SDMA consumes stale descriptors). Interior pads gather
        # row 0 (harmless: their one-hot row is zero via tloc=-1).
        gcnt = np.zeros(len(gathers), dtype=np.int32)
        for gid, (_bb, s0, ln) in enumerate(gathers):
            a = gi[s0:s0 + ln]
            real = np.nonzero(a != -1)[0]
            cut = int(real[-1]) + 1 if len(real) else 0
            head = a[:cut]
            head[head == -1] = 0
            gcnt[gid] = cut

        # wrap gather indices: [16, e/16] tiled x8 -> [128, e/16]
        wrap = np.tile(gi.reshape(e_pad // 16, 16).T, (8, 1)).astype(np.int16)

        import ml_dtypes
        meta = tf.reshape(n_cols, P).T.astype(ml_dtypes.bfloat16)  # [128, C]

        # dis of this core's dest nodes, tiled [128, n_tiles]
        dnode = np.zeros(n_tiles * P, dtype=np.float32)
        dnode[:npc] = dis[c * npc:(c + 1) * npc].astype(np.float32)
        dis_t = dnode.reshape(n_tiles, P).T.copy()  # [128, n_tiles]

        per_core.append({"gidx": wrap, "meta": meta, "dis": dis_t,
                         "gcnt": np.tile(gcnt[None, :], (P, 1))})

    import ml_dtypes
    # wide iota for batched one-hot builds: one tensor_tensor(is_equal)
    # with a stride-0-broadcast tloc operand builds a whole block's
    # one-hot columns in a single DVE instruction.
    max_blk_cols = max(
        sum(len(tile_cols[jj]) for jj in range(
            blk * TILE_BLOCK, min((blk + 1) * TILE_BLOCK, n_tiles)))
        for blk in range(n_blocks))
    iota = np.tile(np.arange(P, dtype=np.float32),
                   (P, max_blk_cols)).astype(ml_dtypes.bfloat16)
    x2b = x2.astype(ml_dtypes.bfloat16)

    sched = {
        "n": n, "d": d, "npc": npc, "n_tiles": n_tiles, "n_cols": n_cols,
        "e_pad": e_pad, "bucket": bucket, "n_buckets": n_buckets,
        "n_blocks": n_blocks, "gathers": gathers, "tile_cols": tile_cols,
        "run_start": run_start, "run_len": run_len,
        "max_blk_cols": max_blk_cols,
    }
    return sched, per_core, x2b, iota


def _build_program(sched, stage="full"):
    n, d, npc = sched["n"], sched["d"], sched["npc"]
    n_tiles, n_cols, e_pad = sched["n_tiles"], sched["n_cols"], sched["e_pad"]
    bucket, n_buckets = sched["bucket"], sched["n_buckets"]
    n_blocks, gathers = sched["n_blocks"], sched["gathers"]
    tile_cols = sched["tile_cols"]

    nc = bacc.Bacc("TRN2", target_bir_lowering=False, debug=False,
                   num_devices=N_CORES, num_swdge_queues=4,
                   dynamic_dma_scratch_size=65536)
    f32 = mybir.dt.float32
    bf16 = mybir.dt.bfloat16
    x_d = nc.dram_tensor("x2", [n, d], bf16, kind="ExternalInput")
    max_blk_cols = sched["max_blk_cols"]
    iota_d = nc.dram_tensor("iota", [P, max_blk_cols * P], bf16,
                            kind="ExternalInput")
    gidx_d = nc.dram_tensor("gidx", [P, e_pad // 16], mybir.dt.int16,
                            kind="ExternalInput")
    meta_d = nc.dram_tensor("meta", [P, n_cols], bf16, kind="ExternalInput")
    dis_d = nc.dram_tensor("dis", [P, n_tiles], f32, kind="ExternalInput")
    gcnt_d = nc.dram_tensor("gcnt", [P, len(gathers)], mybir.dt.int32,
                            kind="ExternalInput")
    y_d = nc.dram_tensor("y", [npc, d], f32, kind="ExternalOutput")

    nc.gpsimd.load_library(mlp_lib)

    # gathers grouped by block for scheduling
    gather_of_col = {}
    for gid, (bb, s0, ln) in enumerate(gathers):
        for cc in range(s0 // P, (s0 + ln) // P):
            gather_of_col[cc] = (gid, s0 // P)

    relu = mybir.ActivationFunctionType.Relu
    act_abs = mybir.ActivationFunctionType.Abs

    with tile.TileContext(nc) as tc:
        with (
            tc.tile_pool(name="const", bufs=1) as cpool,
            tc.tile_pool(name="gather", bufs=28) as gpool,
            tc.tile_pool(name="onehot", bufs=3) as opool,
            tc.tile_pool(name="psum", bufs=8, space="PSUM") as ppool,
            tc.tile_pool(name="outs", bufs=4) as ypool,
        ):
            gidx_t = cpool.tile([P, e_pad // 16], mybir.dt.int16, tag="gidx")
            nc.sync.dma_start(gidx_t[:], gidx_d[:, :])
            iota_t = cpool.tile([P, max_blk_cols * P], bf16, tag="iota")
            nc.sync.dma_start(iota_t[:], iota_d[:, :])
            meta_t = cpool.tile([P, n_cols], bf16, tag="meta")
            nc.sync.dma_start(meta_t[:], meta_d[:, :])
            dis_t = cpool.tile([P, n_tiles], f32, tag="dis")
            nc.sync.dma_start(dis_t[:], dis_d[:, :])
            gcnt_t = cpool.tile([P, len(gathers)], mybir.dt.int32,
                                tag="gcnt")
            nc.sync.dma_start(gcnt_t[:], gcnt_d[:, :])
            REGBATCH = 8
            cnt_regs = [nc.gpsimd.alloc_register(f"gather_cnt{i}")
                        for i in range(REGBATCH)]

            gtiles = {}  # gid -> tile

            def issue_gather(gid):
                bb, s0, ln = gathers[gid]
                base = bb * bucket
                rows = min(bucket, n - base)
                gt = gpool.tile([P, (GATHER_CHUNK // P) * d], bf16, tag="gt",
                                name=f"gt{gid}")
                if gid < 28:
                    # first use of each pool buffer: clear it so slots past
                    # the stripped per-core count read as 0.0, not
                    # uninitialized bits (NaN x 0 = NaN in the matmul).
                    # Later reuses hold stale-but-finite x2 values which
                    # the zero one-hot rows cancel.
                    nc.vector.memset(gt[:], 0.0)
                gt_3d = gt[:, :(ln // P) * d].rearrange("p (c d) -> p c d",
                                                        d=d)
                if gid % REGBATCH == 0:
                    k = min(REGBATCH, len(gathers) - gid)
                    nc.gpsimd.reg_load(cnt_regs[:k],
                                       gcnt_t[0:1, gid:gid + k])
                nc.gpsimd.dma_gather(
                    gt_3d,
                    x_d[base:base + rows, :],
                    gidx_t[:, s0 // 16:(s0 + ln) // 16],
                    ln, cnt_regs[gid % REGBATCH], d,
                    single_packet=True,
                    queue_num=gid % 4,
                )
                gtiles[gid] = gt

            next_gather = 0
            for blk in range(n_blocks):
                tiles_blk = range(blk * TILE_BLOCK,
                                  min((blk + 1) * TILE_BLOCK, n_tiles))
                # issue all gathers needed by this block
                last_col = max(c for jj in tiles_blk for (c, _b) in
                               tile_cols[jj])
                while next_gather < len(gathers):
                    bb, s0, ln = gathers[next_gather]
                    if s0 // P > last_col:
                        break
                    issue_gather(next_gather)
                    next_gather += 1

                blk_cols = [c for jj in tiles_blk for (c, _b) in
                            tile_cols[jj]]
                c_lo, c_hi = min(blk_cols), max(blk_cols) + 1
                nbc = c_hi - c_lo
                if stage != "gather":
                    # one DVE instruction builds the whole block's one-hots:
                    # oh[p, c*128 + f] = (iota[f] == tloc[p, c_lo + c])
                    ohblk = opool.tile([P, max_blk_cols * P], bf16,
                                       tag="ohb", name=f"ohb{blk}")
                    nc.vector.tensor_tensor(
                        ohblk[:, :nbc * P].rearrange(
                            "p (c f) -> p c f", f=P),
                        iota_t[:, :nbc * P].rearrange(
                            "p (c f) -> p c f", f=P),
                        meta_t[:, c_lo:c_hi, None].broadcast_to(
                            [P, nbc, P]),
                        mybir.AluOpType.is_equal,
                    )
                for jj in tiles_blk:
                    cols = tile_cols[jj]
                    rows = min(P, npc - jj * P)
                    yt = ypool.tile([P, d], f32, tag="yt", name=f"yt{jj}")
                    if stage == "gather":
                        gid, col0 = gather_of_col[cols[0][0]]
                        nc.vector.tensor_copy(yt[:],
                                              gtiles[gid][:, :d])
                        nc.sync.dma_start(y_d[jj * P:jj * P + rows, :],
                                          yt[:rows, :])
                        continue
                    pt = ppool.tile([P, d], f32, tag="ps", name=f"ps{jj}")
                    for si, (col, bb) in enumerate(cols):
                        gid, col0 = gather_of_col[col]
                        gt = gtiles[gid]
                        col_l = col - col0
                        nc.tensor.matmul(
                            pt[:],
                            lhsT=ohblk[:, (col - c_lo) * P:
                                       (col - c_lo + 1) * P],
                            rhs=gt[:, col_l * d:(col_l + 1) * d],
                            start=(si == 0),
                            stop=(si == len(cols) - 1),
                        )
                    if stage == "matmul":
                        nc.vector.tensor_copy(yt[:], pt[:])
                    else:
                        nc.scalar.activation(yt[:], pt[:], relu,
                                             scale=dis_t[:, jj:jj + 1])
                    nc.sync.dma_start(y_d[jj * P:jj * P + rows, :],
                                      yt[:rows, :])

    nc.compile()
    return nc


def _run(x, h, t, trace=False, stage="full"):
    import time
    t0 = time.monotonic()
    sched, per_core, x2b, iota = _preprocess(np.asarray(x), np.asarray(h),
                                             np.asarray(t))
    t1 = time.monotonic()
    print(f"[kernel] preprocess {t1 - t0:.1f}s  e_pad={sched['e_pad']} "
          f"cols={sched['n_cols']} gathers={len(sched['gathers'])}",
          flush=True)
    nc = _build_program(sched, stage=stage)
    t2 = time.monotonic()
    print(f"[kernel] build {t2 - t1:.1f}s", flush=True)
    in_maps = [
        {"x2": x2b, "iota": iota, "gidx": pc["gidx"], "meta": pc["meta"],
         "dis": pc["dis"], "gcnt": pc["gcnt"]}
        for pc in per_core
    ]
    res = run_bass_kernel_spmd(nc, in_maps, core_ids=list(range(N_CORES)),
                               trace=trace)
    t3 = time.monotonic()
    print(f"[kernel] compile+run {t3 - t2:.1f}s", flush=True)
    y = np.concatenate([res.results[c]["y"] for c in range(N_CORES)], axis=0)
    return y, res


def kernel(x, h, t):
    y, _ = _run(np.asarray(x), np.asarray(h), np.asarray(t))
    return y


# revision 23
# speedup vs baseline: 1.0980x; 1.0980x over previous
"""PlainGCN message passing on 8 TRN2 NeuronCores.

Computation (reference):
    deg = bincount(h); dis = deg**-0.5; norm = dis[t]*dis[h]
    out = relu(segment_sum(norm[:,None] * x[h], t, N))

Strategy (v2):
  - Fold dis[h] into x host-side: x2 = dis[:,None]*x (bf16). Then
    out[t] = relu(dis[t] * segment_sum(x2[h], t)) — the per-edge norm
    disappears; dis[t] is applied once per dest tile, fused with the
    ReLU on ScalarE (per-partition scale).
  - Shard edges by destination: core c owns dest nodes
    [c*N/8, (c+1)*N/8); x2 replicated in HBM.
  - Per (dest tile j, source bucket b) the edges form a run padded to
    a multiple of 128 slots (shared SPMD schedule = max over cores).
    Pad slots carry gather idx -1 (skipped by the DMA when trailing)
    and tloc -1 (one-hot row = 0).
  - dma_gather x2 rows (256 B bf16) in <=1024-idx chunks,
    single_packet=True, round-robin over 4 SWDGE queues. The gather is
    Q7 descriptor-generation bound (~4 ns/idx), so everything else is
    scheduled to hide under it.
  - Segment-sum per dest tile: one-hot(tloc)=is_equal(iota) on DVE
    (bf16), TensorE matmul accumulation of full 128-slot columns into
    one PSUM bank per tile, then Relu(dis_t * psum) on ScalarE, DMA out.
"""

import numpy as np

import concourse.bacc as bacc
import concourse.mybir as mybir
import concourse.tile as tile
from concourse.bass_utils import run_bass_kernel_spmd
from concourse.library_config import mlp as mlp_lib

P = 128
N_NODES = 100000
D_FEAT = 128
N_CORES = 8
BUCKET_W = 25000     # source-bucket width (< 32768 so idx fits int16)
TILE_BLOCK = 4       # dest tiles per gather block
GATHER_CHUNK = 1024  # max idxs per dma_gather (single_packet safe limit)


def _preprocess(x, h, t):
    n, d = x.shape
    assert (n, d) == (N_NODES, D_FEAT)
    npc = n // N_CORES
    n_tiles = -(-npc // P)
    bucket = BUCKET_W
    n_buckets = -(-n // bucket)

    h = h.astype(np.int64)
    t = t.astype(np.int64)

    deg = np.bincount(h, minlength=n).astype(np.float64)
    dis = np.where(deg > 0, deg, 1.0) ** -0.5
    x2 = (x.astype(np.float64) * dis[:, None]).astype(np.float32)

    core = t // npc
    tloc = t - core * npc
    j = tloc // P
    tin = (tloc % P).astype(np.float32)
    b = h // BUCKET_W
    gidx_all = (h - b * BUCKET_W).astype(np.int16)

    counts = np.zeros((N_CORES, n_tiles, n_buckets), dtype=np.int64)
    np.add.at(counts, (core, j, b), 1)
    run_len = counts.max(axis=0)
    run_len = -(-run_len // P) * P  # full 128-slot columns only

    # stream order: blocks of TILE_BLOCK dest tiles; per block, per bucket,
    # the tiles' runs back to back.
    n_blocks = -(-n_tiles // TILE_BLOCK)
    run_start = np.zeros((n_tiles, n_buckets), dtype=np.int64)
    gathers = []  # (bucket, start, length) — <=GATHER_CHUNK, 128-aligned
    pos = 0
    for blk in range(n_blocks):
        tiles_blk = range(blk * TILE_BLOCK, min((blk + 1) * TILE_BLOCK, n_tiles))
        for bb in range(n_buckets):
            s0 = pos
            for jj in tiles_blk:
                run_start[jj, bb] = pos
                pos += int(run_len[jj, bb])
            # split [s0, pos) into gather chunks at run boundaries
            c0 = s0
            for jj in tiles_blk:
                r = int(run_len[jj, bb])
                end = run_start[jj, bb] + r
                if end - c0 > GATHER_CHUNK:
                    if run_start[jj, bb] > c0:
                        gathers.append((bb, c0, int(run_start[jj, bb] - c0)))
                    c0 = int(run_start[jj, bb])
                    while end - c0 > GATHER_CHUNK:
                        gathers.append((bb, c0, GATHER_CHUNK))
                        c0 += GATHER_CHUNK
            if pos > c0:
                gathers.append((bb, c0, int(pos - c0)))
    e_pad = pos
    n_cols = e_pad // P
    assert all(ln <= GATHER_CHUNK and ln % P == 0 for (_b, _s, ln) in gathers)

    # per-tile column lists: (col, bucket)
    tile_cols = []
    for jj in range(n_tiles):
        cols = []
        for bb in range(n_buckets):
            s, r = int(run_start[jj, bb]), int(run_len[jj, bb])
            cols.extend((c, bb) for c in range(s // P, (s + r) // P))
        tile_cols.append(cols)

    # per-core streams
    order_key = (j // TILE_BLOCK) * (n_buckets * n_tiles) + b * n_tiles + j
    per_core = []
    for c in range(N_CORES):
        sel = np.nonzero(core == c)[0]
        sel = sel[np.argsort(order_key[sel], kind="stable")]
        jj = j[sel]
        bb2 = b[sel]
        key = jj * n_buckets + bb2
        change = np.r_[True, key[1:] != key[:-1]]
        grp_id = np.cumsum(change) - 1
        first_pos = np.nonzero(change)[0]
        within = np.arange(len(sel)) - first_pos[grp_id]
        posn = run_start[jj, bb2] + within

        gi = np.full(e_pad, -1, dtype=np.int16)
        tf = np.full(e_pad, -1.0, dtype=np.float32)
        gi[posn] = gidx_all[sel]
        tf[posn] = tin[sel]
        # pad slots gather row 0 (harmless: their one-hot row is zero via
        # tloc=-1). Runtime -1/register count stripping was measured to
        # cost more in SEQ serialization + first-use memsets than the Q7
        # generation it saves — pads are simply gathered.
        gi[gi == -1] = 0

        # wrap gather indices: [16, e/16] tiled x8 -> [128, e/16]
        wrap = np.tile(gi.reshape(e_pad // 16, 16).T, (8, 1)).astype(np.int16)

        import ml_dtypes
        meta = tf.reshape(n_cols, P).T.astype(ml_dtypes.bfloat16)  # [128, C]

        # dis of this core's dest nodes, tiled [128, n_tiles]
        dnode = np.zeros(n_tiles * P, dtype=np.float32)
        dnode[:npc] = dis[c * npc:(c + 1) * npc].astype(np.float32)
        dis_t = dnode.reshape(n_tiles, P).T.copy()  # [128, n_tiles]

        per_core.append({"gidx": wrap, "meta": meta, "dis": dis_t})

    import ml_dtypes
    # wide iota for batched one-hot builds: one tensor_tensor(is_equal)
    # with a stride-0-broadcast tloc operand builds a whole block's
    # one-hot columns in a single DVE instruction.
    max_blk_cols = max(
        sum(len(tile_cols[jj]) for jj in range(
            blk * TILE_BLOCK, min((blk + 1) * TILE_BLOCK, n_tiles)))
        for blk in range(n_blocks))
    iota = np.tile(np.arange(P, dtype=np.float32),
                   (P, max_blk_cols)).astype(ml_dtypes.bfloat16)
    x2b = x2.astype(ml_dtypes.bfloat16)

    sched = {
        "n": n, "d": d, "npc": npc, "n_tiles": n_tiles, "n_cols": n_cols,
        "e_pad": e_pad, "bucket": bucket, "n_buckets": n_buckets,
        "n_blocks": n_blocks, "gathers": gathers, "tile_cols": tile_cols,
        "run_start": run_start, "run_len": run_len,
        "max_blk_cols": max_blk_cols,
    }
    return sched, per_core, x2b, iota


def _build_program(sched, stage="full"):
    n, d, npc = sched["n"], sched["d"], sched["npc"]
    n_tiles, n_cols, e_pad = sched["n_tiles"], sched["n_cols"], sched["e_pad"]
    bucket, n_buckets = sched["bucket"], sched["n_buckets"]
    n_blocks, gathers = sched["n_blocks"], sched["gathers"]
    tile_cols = sched["tile_cols"]

    nc = bacc.Bacc("TRN2", target_bir_lowering=False, debug=False,
                   num_devices=N_CORES, num_swdge_queues=4,
                   dynamic_dma_scratch_size=65536)
    f32 = mybir.dt.float32
    bf16 = mybir.dt.bfloat16
    x_d = nc.dram_tensor("x2", [n, d], bf16, kind="ExternalInput")
    max_blk_cols = sched["max_blk_cols"]
    iota_d = nc.dram_tensor("iota", [P, max_blk_cols * P], bf16,
                            kind="ExternalInput")
    gidx_d = nc.dram_tensor("gidx", [P, e_pad // 16], mybir.dt.int16,
                            kind="ExternalInput")
    meta_d = nc.dram_tensor("meta", [P, n_cols], bf16, kind="ExternalInput")
    dis_d = nc.dram_tensor("dis", [P, n_tiles], f32, kind="ExternalInput")
    y_d = nc.dram_tensor("y", [npc, d], f32, kind="ExternalOutput")

    nc.gpsimd.load_library(mlp_lib)

    # gathers grouped by block for scheduling
    gather_of_col = {}
    for gid, (bb, s0, ln) in enumerate(gathers):
        for cc in range(s0 // P, (s0 + ln) // P):
            gather_of_col[cc] = (gid, s0 // P)

    relu = mybir.ActivationFunctionType.Relu
    act_abs = mybir.ActivationFunctionType.Abs

    with tile.TileContext(nc) as tc:
        with (
            tc.tile_pool(name="const", bufs=1) as cpool,
            tc.tile_pool(name="gather", bufs=28) as gpool,
            tc.tile_pool(name="onehot", bufs=3) as opool,
            tc.tile_pool(name="psum", bufs=8, space="PSUM") as ppool,
            tc.tile_pool(name="outs", bufs=4) as ypool,
        ):
            gidx_t = cpool.tile([P, e_pad // 16], mybir.dt.int16, tag="gidx")
            nc.sync.dma_start(gidx_t[:], gidx_d[:, :])
            iota_t = cpool.tile([P, max_blk_cols * P], bf16, tag="iota")
            nc.sync.dma_start(iota_t[:], iota_d[:, :])
            meta_t = cpool.tile([P, n_cols], bf16, tag="meta")
            nc.sync.dma_start(meta_t[:], meta_d[:, :])
            dis_t = cpool.tile([P, n_tiles], f32, tag="dis")
            nc.sync.dma_start(dis_t[:], dis_d[:, :])

            gtiles = {}  # gid -> tile

            def issue_gather(gid):
                bb, s0, ln = gathers[gid]
                base = bb * bucket
                rows = min(bucket, n - base)
                gt = gpool.tile([P, (GATHER_CHUNK // P) * d], bf16, tag="gt",
                                name=f"gt{gid}")
                gt_3d = gt[:, :(ln // P) * d].rearrange("p (c d) -> p c d",
                                                        d=d)
                nc.gpsimd.dma_gather(
                    gt_3d,
                    x_d[base:base + rows, :],
                    gidx_t[:, s0 // 16:(s0 + ln) // 16],
                    ln, ln, d,
                    single_packet=True,
                    queue_num=gid % 4,
                )
                gtiles[gid] = gt

            next_gather = 0
            for blk in range(n_blocks):
                tiles_blk = range(blk * TILE_BLOCK,
                                  min((blk + 1) * TILE_BLOCK, n_tiles))
                # issue all gathers needed by this block
                last_col = max(c for jj in tiles_blk for (c, _b) in
                               tile_cols[jj])
                while next_gather < len(gathers):
                    bb, s0, ln = gathers[next_gather]
                    if s0 // P > last_col:
                        break
                    issue_gather(next_gather)
                    next_gather += 1

                blk_cols = [c for jj in tiles_blk for (c, _b) in
                            tile_cols[jj]]
                c_lo, c_hi = min(blk_cols), max(blk_cols) + 1
                nbc = c_hi - c_lo
                if stage != "gather":
                    # one DVE instruction builds the whole block's one-hots:
                    # oh[p, c*128 + f] = (iota[f] == tloc[p, c_lo + c])
                    ohblk = opool.tile([P, max_blk_cols * P], bf16,
                                       tag="ohb", name=f"ohb{blk}")
                    nc.vector.tensor_tensor(
                        ohblk[:, :nbc * P].rearrange(
                            "p (c f) -> p c f", f=P),
                        iota_t[:, :nbc * P].rearrange(
                            "p (c f) -> p c f", f=P),
                        meta_t[:, c_lo:c_hi, None].broadcast_to(
                            [P, nbc, P]),
                        mybir.AluOpType.is_equal,
                    )
                for jj in tiles_blk:
                    cols = tile_cols[jj]
                    rows = min(P, npc - jj * P)
                    yt = ypool.tile([P, d], f32, tag="yt", name=f"yt{jj}")
                    if stage == "gather":
                        gid, col0 = gather_of_col[cols[0][0]]
                        nc.vector.tensor_copy(yt[:],
                                              gtiles[gid][:, :d])
                        nc.sync.dma_start(y_d[jj * P:jj * P + rows, :],
                                          yt[:rows, :])
                        continue
                    pt = ppool.tile([P, d], f32, tag="ps", name=f"ps{jj}")
                    for si, (col, bb) in enumerate(cols):
                        gid, col0 = gather_of_col[col]
                        gt = gtiles[gid]
                        col_l = col - col0
                        nc.tensor.matmul(
                            pt[:],
                            lhsT=ohblk[:, (col - c_lo) * P:
                                       (col - c_lo + 1) * P],
                            rhs=gt[:, col_l * d:(col_l + 1) * d],
                            start=(si == 0),
                            stop=(si == len(cols) - 1),
                        )
                    if stage == "matmul":
                        nc.vector.tensor_copy(yt[:], pt[:])
                    else:
                        nc.scalar.activation(yt[:], pt[:], relu,
                                             scale=dis_t[:, jj:jj + 1])
                    nc.sync.dma_start(y_d[jj * P:jj * P + rows, :],
                                      yt[:rows, :])

    nc.compile()
    return nc


def _run(x, h, t, trace=False, stage="full"):
    import time
    t0 = time.monotonic()
    sched, per_core, x2b, iota = _preprocess(np.asarray(x), np.asarray(h),
                                             np.asarray(t))
    t1 = time.monotonic()
    print(f"[kernel] preprocess {t1 - t0:.1f}s  e_pad={sched['e_pad']} "
          f"cols={sched['n_cols']} gathers={len(sched['gathers'])}",
          flush=True)
    nc = _build_program(sched, stage=stage)
    t2 = time.monotonic()
    print(f"[kernel] build {t2 - t1:.1f}s", flush=True)
    in_maps = [
        {"x2": x2b, "iota": iota, "gidx": pc["gidx"], "meta": pc["meta"],
         "dis": pc["dis"]}
        for pc in per_core
    ]
    res = run_bass_kernel_spmd(nc, in_maps, core_ids=list(range(N_CORES)),
                               trace=trace)
    t3 = time.monotonic()
    print(f"[kernel] compile+run {t3 - t2:.1f}s", flush=True)
    y = np.concatenate([res.results[c]["y"] for c in range(N_CORES)], axis=0)
    return y, res


def kernel(x, h, t):
    y, _ = _run(np.asarray(x), np.asarray(h), np.asarray(t))
    return y
